# revision 24
# baseline (speedup 1.0000x reference)
"""Trainium2 Bass kernel for nn_Attention_10307921511133.

Full-input contract: kernel(**inputs) takes the complete unsharded tensors
(query/key/value [8, 2048, 1024] f32, mask [8, 2048] i32, Wq_w/Wk_w [1024,
1024] f32, Wq_b/Wk_b [1024] f32) and returns the full [8, 2048, 1024] f32
output.

Sharding: data-parallel over batch. B == n_cores == 8, so each NeuronCore
processes one batch element end-to-end; no collectives.

Per-core algorithm (L=2048 rows, D=1024 features):
  1. PE-transpose Q and K input tiles (cast to bf16 during DMA) to get
     QT/KT in [d, i] layout (contraction dim on partitions).
  2. Projections as qT = WqT.T @ QT -> qT/kT in [e, i] layout.
  3. S^T strips: S^T[j,:] = kT[:,j].T @ qT, then P^T = exp(S^T/32 + bias_j)
     on ScalarE, where bias_j = (mask_j - 1) * 50 folds the multiplicative+
     additive masking into the softmax (masked logits get -50 => exp ~ 1e-22,
     negligible vs kept terms, identical to the reference's -1e9 path).
  4. O = P^T.T @ V accumulated over j-tiles in PSUM; the softmax denominator
     comes from a third matmul against a ones-vector sharing the same
     stationary weights; final normalize fused into the PSUM->SBUF copy.
"""

import sys

if "/opt/trn_rl_repo" not in sys.path:
    sys.path.insert(0, "/opt/trn_rl_repo")

import numpy as np

B = 8
L = 2048
D = 1024
LT = L // 128   # 16 row tiles
DT = D // 128   # 8 feature tiles
N_CORES = 8
SCALE = 1.0 / 32.0      # 1/sqrt(D)
MASK_BIAS = -50.0       # added to masked logits before exp

_cache: dict = {}


def _build_program():
    import concourse.bass as bass
    import concourse.mybir as mybir
    import concourse.tile as tile
    from concourse import bacc
    from concourse.masks import make_identity

    f32 = mybir.dt.float32
    bf16 = mybir.dt.float16  # fp16: same PE speed as bf16, 8x the mantissa
    i32 = mybir.dt.int32
    AF = mybir.ActivationFunctionType
    ALU = mybir.AluOpType

    nc = bacc.Bacc("TRN2", target_bir_lowering=False, debug=False,
                   num_devices=N_CORES)

    q_d = nc.dram_tensor("query", [L, D], f32, kind="ExternalInput")
    k_d = nc.dram_tensor("key", [L, D], f32, kind="ExternalInput")
    v_d = nc.dram_tensor("value", [L, D], f32, kind="ExternalInput")
    m_d = nc.dram_tensor("mask", [L], i32, kind="ExternalInput")
    wq_d = nc.dram_tensor("Wq_w", [D, D], f32, kind="ExternalInput")
    wqb_d = nc.dram_tensor("Wq_b", [D], f32, kind="ExternalInput")
    wk_d = nc.dram_tensor("Wk_w", [D, D], f32, kind="ExternalInput")
    wkb_d = nc.dram_tensor("Wk_b", [D], f32, kind="ExternalInput")
    o_d = nc.dram_tensor("out", [L, D], f32, kind="ExternalOutput")

    with tile.TileContext(nc) as tc:
        singles = tc.alloc_tile_pool(name="singles", bufs=1)
        psum = tc.alloc_tile_pool(name="psum", bufs=1, space="PSUM")
        stage = tc.alloc_tile_pool(name="stage", bufs=1)

        # ---- small constants -------------------------------------------
        ident_bf = singles.tile([128, 128], bf16, name="ident_bf")
        make_identity(nc, ident_bf)
        ident_f32 = singles.tile([128, 128], f32, name="ident_f32")
        make_identity(nc, ident_f32)
        ones_bf = singles.tile([128, 1], bf16, name="ones_bf")
        nc.vector.memset(ones_bf, 1.0)

        # mask -> per-partition exp bias, laid out [128 j_in_tile, 16 jt]
        m_i32 = singles.tile([16, 128], i32, name="m_i32")
        nc.sync.dma_start(out=m_i32, in_=m_d.ap().rearrange("(t p) -> t p", p=128))
        m_f = singles.tile([16, 128], f32, name="m_f")
        nc.vector.tensor_copy(out=m_f, in_=m_i32)
        mt_ps = psum.tile([128, 16], f32, tag="mm", bufs=6, name="mt_ps")
        nc.tensor.transpose(mt_ps, m_f, ident_f32[0:16, 0:16])
        bias_b = singles.tile([128, LT], f32, name="bias_b")
        nc.vector.tensor_scalar(out=bias_b, in0=mt_ps, scalar1=-MASK_BIAS,
                                scalar2=MASK_BIAS, op0=ALU.mult, op1=ALU.add)

        # linear biases, laid out [128 e_in_tile, 8 et]
        def load_bias(bdram, name):
            b_nat = singles.tile([DT, 128], f32, name=f"{name}_nat")
            nc.sync.dma_start(out=b_nat,
                              in_=bdram.ap().rearrange("(t p) -> t p", p=128))
            b_ps = psum.tile([128, DT], f32, tag="mm", bufs=6, name=f"{name}_ps")
            nc.tensor.transpose(b_ps, b_nat, ident_f32[0:DT, 0:DT])
            b_sb = singles.tile([128, DT], f32, name=name)
            nc.vector.tensor_copy(out=b_sb, in_=b_ps)
            return b_sb

        qb_sb = load_bias(wqb_d, "qb_sb")
        kb_sb = load_bias(wkb_d, "kb_sb")

        # ---- stage A: weights -> WT [d, e] bf16 ------------------------
        pool_w = tc.alloc_tile_pool(name="pool_w", bufs=1)
        wqT = pool_w.tile([128, DT, D], bf16, name="wqT")
        wkT = pool_w.tile([128, DT, D], bf16, name="wkT")

        def transpose_in(dst, src_dram, row0, n_rows, n_rows_name):
            """Load n_rows natural row-tiles [128, D] with ONE SWDGE DMA
            (f32->bf16 cast during the transfer; one descriptor-generation
            pass on the Q7 instead of n_rows) and PE-transpose each
            [128,128] block into dst[:, dt, row*128:...]."""
            nat = stage.tile([128, n_rows, D], bf16, tag="nat", bufs=5,
                             name=f"nat_{n_rows_name}_{row0}")
            src = src_dram[row0 * 128:(row0 + n_rows) * 128, :]
            nc.gpsimd.dma_start(
                out=nat, in_=src.rearrange("(t p) d -> p t d", p=128))
            for r in range(n_rows):
                row = row0 + r
                for half in range(DT // 4):
                    tr = psum.tile([128, 4, 128], bf16, tag="mm", bufs=6,
                                   name=f"tr_{n_rows_name}_{row}_{half}")
                    for g in range(4):
                        dt_ = half * 4 + g
                        nc.tensor.transpose(
                            tr[:, g, :],
                            nat[:, r, dt_ * 128:(dt_ + 1) * 128],
                            ident_bf)
                    nc.any.tensor_copy(
                        out=dst[:, half * 4:(half + 1) * 4,
                                row * 128:(row + 1) * 128],
                        in_=tr)

        # ---- stages A-C: per tensor X in {Q, K}: transpose W_x, transpose
        # X, project x. Projections are split into two i-halves so the first
        # half's matmuls start after only half of X has streamed in; K's
        # loads and transposes overlap the q projection.
        pool_xt = tc.alloc_tile_pool(name="pool_xt", bufs=1)
        QT = pool_xt.tile([128, DT, L], bf16, name="QT")
        KT = pool_xt.tile([128, DT, L], bf16, name="KT")
        pool_qkt = tc.alloc_tile_pool(name="pool_qkt", bufs=1, side="right")
        qT = pool_qkt.tile([128, DT, L], bf16, name="qT")
        kT = pool_qkt.tile([128, DT, L], bf16, name="kT")

        def project_half(dst, wT, xT, b_sb, nm, hf):
            for et in range(DT):
                pss = [psum.tile([128, 512], f32, tag="mm", bufs=6,
                                 name=f"p_{nm}_{hf}_{et}_{ic}")
                       for ic in range(2)]
                for dt_ in range(DT):
                    lhs = wT[:, dt_, et * 128:(et + 1) * 128]
                    for ic in range(2):
                        nc.tensor.matmul(
                            pss[ic], lhs,
                            xT[:, dt_, (2 * hf + ic) * 512:
                               (2 * hf + ic + 1) * 512],
                            start=(dt_ == 0), stop=(dt_ == DT - 1))
                for ic in range(2):
                    nc.any.tensor_scalar_add(
                        out=dst[:, et, (2 * hf + ic) * 512:
                                (2 * hf + ic + 1) * 512],
                        in0=pss[ic], scalar1=b_sb[:, et:et + 1])

        for w_d_, wT_, x_d_, xT_, xt_sb_, b_sb_, nm_ in (
                (wq_d, wqT, q_d, QT, qT, qb_sb, "q"),
                (wk_d, wkT, k_d, KT, kT, kb_sb, "k")):
            for et in range(0, DT, 2):
                transpose_in(wT_, w_d_, et, 2, f"w{nm_}")
            for it in range(0, LT // 2, 2):
                transpose_in(xT_, x_d_, it, 2, nm_)
            project_half(xt_sb_, wT_, xT_, b_sb_, nm_, 0)
            for it in range(LT // 2, LT, 2):
                transpose_in(xT_, x_d_, it, 2, nm_)
            project_half(xt_sb_, wT_, xT_, b_sb_, nm_, 1)
        pool_xt.release()
        pool_w.release()

        # ---- V load (overlaps stage D) ---------------------------------
        pool_v = tc.alloc_tile_pool(name="pool_v", bufs=1)
        V_sb = pool_v.tile([128, LT, D], bf16, name="V_sb")
        for jt in range(LT):
            nc.gpsimd.dma_start(out=V_sb[:, jt, :],
                                in_=v_d[jt * 128:(jt + 1) * 128, :])

        # ---- stage D: S^T strips + exp -> P^T [j, i] bf16 --------------
        pool_pt = tc.alloc_tile_pool(name="pool_pt", bufs=1)
        PT = pool_pt.tile([128, LT, L], bf16, name="PT")
        for jt in range(LT):
            pss = [psum.tile([128, 512], f32, tag="mm", bufs=6,
                             name=f"s_{jt}_{ic}") for ic in range(4)]
            for et in range(DT):
                lhs = kT[:, et, jt * 128:(jt + 1) * 128]
                for ic in range(4):
                    nc.tensor.matmul(pss[ic], lhs,
                                     qT[:, et, ic * 512:(ic + 1) * 512],
                                     start=(et == 0), stop=(et == DT - 1))
            for ic in range(4):
                nc.scalar.activation(out=PT[:, jt, ic * 512:(ic + 1) * 512],
                                     in_=pss[ic], func=AF.Exp,
                                     bias=bias_b[:, jt:jt + 1], scale=SCALE)
        pool_qkt.release()

        # ---- stage E: O = P^T.T @ V, denominator, normalize, store -----
        for it in range(LT):
            o_lo = psum.tile([128, 512], f32, tag="mm", bufs=6,
                             name=f"o_lo_{it}")
            o_hi = psum.tile([128, 512], f32, tag="mm", bufs=6,
                             name=f"o_hi_{it}")
            d_ps = psum.tile([128, 1], f32, tag="d", bufs=2,
                             name=f"d_ps_{it}")
            for jt in range(LT):
                lhs = PT[:, jt, it * 128:(it + 1) * 128]
                # denominator first: its matmul is tiny, so the (hidden)
                # weight load is immediately followed by dense N=512 work
                nc.tensor.matmul(d_ps, lhs, ones_bf,
                                 start=(jt == 0), stop=(jt == LT - 1))
                nc.tensor.matmul(o_lo, lhs, V_sb[:, jt, 0:512],
                                 start=(jt == 0), stop=(jt == LT - 1))
                nc.tensor.matmul(o_hi, lhs, V_sb[:, jt, 512:1024],
                                 start=(jt == 0), stop=(jt == LT - 1))
            recip = stage.tile([128, 1], f32, tag="recip", bufs=4,
                               name=f"recip_{it}")
            nc.vector.reciprocal(recip, d_ps)
            o_sb = stage.tile([128, D], f32, tag="osb", bufs=3,
                              name=f"o_sb_{it}")
            nc.any.tensor_scalar_mul(o_sb[:, 0:512], o_lo, recip)
            nc.any.tensor_scalar_mul(o_sb[:, 512:1024], o_hi, recip)
            nc.sync.dma_start(out=o_d[it * 128:(it + 1) * 128, :], in_=o_sb)
        pool_pt.release()
        pool_v.release()

        stage.release()
        psum.release()
        singles.release()

    nc.compile()
    return nc


def _get_program():
    if "nc" not in _cache:
        _cache["nc"] = _build_program()
    return _cache["nc"]


def _enable_jax_cache():
    try:
        import jax
        jax.config.update("jax_compilation_cache_dir",
                          "/tmp/jax_neff_cache")
        jax.config.update("jax_persistent_cache_min_compile_time_secs", 10.0)
        jax.config.update("jax_persistent_cache_min_entry_size_bytes", -1)
    except Exception:
        pass


def kernel(query, key, value, mask, Wq_w, Wq_b, Wk_w, Wk_b):
    from concourse.bass_utils import run_bass_kernel_spmd

    _enable_jax_cache()
    nc = _get_program()
    in_maps = []
    for c in range(N_CORES):
        in_maps.append({
            "query": np.ascontiguousarray(query[c], dtype=np.float32),
            "key": np.ascontiguousarray(key[c], dtype=np.float32),
            "value": np.ascontiguousarray(value[c], dtype=np.float32),
            "mask": np.ascontiguousarray(mask[c], dtype=np.int32),
            "Wq_w": np.ascontiguousarray(Wq_w, dtype=np.float32),
            "Wq_b": np.ascontiguousarray(Wq_b, dtype=np.float32),
            "Wk_w": np.ascontiguousarray(Wk_w, dtype=np.float32),
            "Wk_b": np.ascontiguousarray(Wk_b, dtype=np.float32),
        })
    res = run_bass_kernel_spmd(nc, in_maps, list(range(N_CORES)))
    return np.stack([res.results[c]["out"] for c in range(N_CORES)], axis=0)


# revision 27
# speedup vs baseline: 1.0612x; 1.0612x over previous
"""Trainium2 Bass kernel for nn_Attention_10307921511133.

Full-input contract: kernel(**inputs) takes the complete unsharded tensors
(query/key/value [8, 2048, 1024] f32, mask [8, 2048] i32, Wq_w/Wk_w [1024,
1024] f32, Wq_b/Wk_b [1024] f32) and returns the full [8, 2048, 1024] f32
output.

Sharding: data-parallel over batch. B == n_cores == 8, so each NeuronCore
processes one batch element end-to-end; no collectives.

Per-core algorithm (L=2048 rows, D=1024 features):
  1. PE-transpose Q and K input tiles (cast to bf16 during DMA) to get
     QT/KT in [d, i] layout (contraction dim on partitions).
  2. Projections as qT = WqT.T @ QT -> qT/kT in [e, i] layout.
  3. S^T strips: S^T[j,:] = kT[:,j].T @ qT, then P^T = exp(S^T/32 + bias_j)
     on ScalarE, where bias_j = (mask_j - 1) * 50 folds the multiplicative+
     additive masking into the softmax (masked logits get -50 => exp ~ 1e-22,
     negligible vs kept terms, identical to the reference's -1e9 path).
  4. O = P^T.T @ V accumulated over j-tiles in PSUM; the softmax denominator
     comes from a third matmul against a ones-vector sharing the same
     stationary weights; final normalize fused into the PSUM->SBUF copy.
"""

import sys

if "/opt/trn_rl_repo" not in sys.path:
    sys.path.insert(0, "/opt/trn_rl_repo")

import numpy as np

B = 8
L = 2048
D = 1024
LT = L // 128   # 16 row tiles
DT = D // 128   # 8 feature tiles
N_CORES = 8
SCALE = 1.0 / 32.0      # 1/sqrt(D)
MASK_BIAS = -50.0       # added to masked logits before exp

_cache: dict = {}


def _build_program():
    import concourse.bass as bass
    import concourse.mybir as mybir
    import concourse.tile as tile
    from concourse import bacc
    from concourse.masks import make_identity

    f32 = mybir.dt.float32
    bf16 = mybir.dt.float16  # fp16: same PE speed as bf16, 8x the mantissa
    i32 = mybir.dt.int32
    AF = mybir.ActivationFunctionType
    ALU = mybir.AluOpType

    nc = bacc.Bacc("TRN2", target_bir_lowering=False, debug=False,
                   num_devices=N_CORES)

    q_d = nc.dram_tensor("query", [L, D], f32, kind="ExternalInput")
    k_d = nc.dram_tensor("key", [L, D], f32, kind="ExternalInput")
    v_d = nc.dram_tensor("value", [L, D], f32, kind="ExternalInput")
    m_d = nc.dram_tensor("mask", [L], i32, kind="ExternalInput")
    wq_d = nc.dram_tensor("Wq_w", [D, D], f32, kind="ExternalInput")
    wqb_d = nc.dram_tensor("Wq_b", [D], f32, kind="ExternalInput")
    wk_d = nc.dram_tensor("Wk_w", [D, D], f32, kind="ExternalInput")
    wkb_d = nc.dram_tensor("Wk_b", [D], f32, kind="ExternalInput")
    o_d = nc.dram_tensor("out", [L, D], f32, kind="ExternalOutput")

    with tile.TileContext(nc) as tc:
        singles = tc.alloc_tile_pool(name="singles", bufs=1)
        psum = tc.alloc_tile_pool(name="psum", bufs=1, space="PSUM")
        stage = tc.alloc_tile_pool(name="stage", bufs=1)

        # ---- small constants -------------------------------------------
        ident_bf = singles.tile([128, 128], bf16, name="ident_bf")
        make_identity(nc, ident_bf)
        ident_f32 = singles.tile([128, 128], f32, name="ident_f32")
        make_identity(nc, ident_f32)
        ones_bf = singles.tile([128, 1], bf16, name="ones_bf")
        nc.vector.memset(ones_bf, 1.0)

        # mask -> per-partition exp bias, laid out [128 j_in_tile, 16 jt]
        m_i32 = singles.tile([16, 128], i32, name="m_i32")
        nc.sync.dma_start(out=m_i32, in_=m_d.ap().rearrange("(t p) -> t p", p=128))
        m_f = singles.tile([16, 128], f32, name="m_f")
        nc.vector.tensor_copy(out=m_f, in_=m_i32)
        mt_ps = psum.tile([128, 16], f32, tag="mm", bufs=6, name="mt_ps")
        nc.tensor.transpose(mt_ps, m_f, ident_f32[0:16, 0:16])
        bias_b = singles.tile([128, LT], f32, name="bias_b")
        nc.vector.tensor_scalar(out=bias_b, in0=mt_ps, scalar1=-MASK_BIAS,
                                scalar2=MASK_BIAS, op0=ALU.mult, op1=ALU.add)

        # linear biases, laid out [128 e_in_tile, 8 et]
        def load_bias(bdram, name):
            b_nat = singles.tile([DT, 128], f32, name=f"{name}_nat")
            nc.sync.dma_start(out=b_nat,
                              in_=bdram.ap().rearrange("(t p) -> t p", p=128))
            b_ps = psum.tile([128, DT], f32, tag="mm", bufs=6, name=f"{name}_ps")
            nc.tensor.transpose(b_ps, b_nat, ident_f32[0:DT, 0:DT])
            b_sb = singles.tile([128, DT], f32, name=name)
            nc.vector.tensor_copy(out=b_sb, in_=b_ps)
            return b_sb

        qb_sb = load_bias(wqb_d, "qb_sb")
        kb_sb = load_bias(wkb_d, "kb_sb")

        # ---- stage A: weights -> WT [d, e] bf16 ------------------------
        pool_w = tc.alloc_tile_pool(name="pool_w", bufs=1)
        wqT = pool_w.tile([128, DT, D], bf16, name="wqT")
        wkT = pool_w.tile([128, DT, D], bf16, name="wkT")

        def transpose_in(dst, src_dram, row0, n_rows, n_rows_name):
            """Load n_rows natural row-tiles [128, D] with ONE SWDGE DMA
            (f32->bf16 cast during the transfer; one descriptor-generation
            pass on the Q7 instead of n_rows) and PE-transpose each
            [128,128] block into dst[:, dt, row*128:...]."""
            nat = stage.tile([128, n_rows, D], bf16, tag="nat", bufs=6,
                             name=f"nat_{n_rows_name}_{row0}")
            src = src_dram[row0 * 128:(row0 + n_rows) * 128, :]
            nc.gpsimd.dma_start(
                out=nat, in_=src.rearrange("(t p) d -> p t d", p=128))
            for r in range(n_rows):
                row = row0 + r
                for half in range(DT // 4):
                    tr = psum.tile([128, 4, 128], bf16, tag="mm", bufs=6,
                                   name=f"tr_{n_rows_name}_{row}_{half}")
                    for g in range(4):
                        dt_ = half * 4 + g
                        nc.tensor.transpose(
                            tr[:, g, :],
                            nat[:, r, dt_ * 128:(dt_ + 1) * 128],
                            ident_bf)
                    nc.any.tensor_copy(
                        out=dst[:, half * 4:(half + 1) * 4,
                                row * 128:(row + 1) * 128],
                        in_=tr)

        # ---- stages A-C: per tensor X in {Q, K}: transpose W_x, transpose
        # X, project x. Projections are split into two i-halves so the first
        # half's matmuls start after only half of X has streamed in; K's
        # loads and transposes overlap the q projection.
        pool_xt = tc.alloc_tile_pool(name="pool_xt", bufs=1)
        QT = pool_xt.tile([128, DT, L], bf16, name="QT")
        KT = pool_xt.tile([128, DT, L], bf16, name="KT")
        pool_qkt = tc.alloc_tile_pool(name="pool_qkt", bufs=1, side="right")
        qT = pool_qkt.tile([128, DT, L], bf16, name="qT")
        kT = pool_qkt.tile([128, DT, L], bf16, name="kT")

        def project_half(dst, wT, xT, b_sb, nm, hf):
            for et in range(DT):
                pss = [psum.tile([128, 512], f32, tag="mm", bufs=6,
                                 name=f"p_{nm}_{hf}_{et}_{ic}")
                       for ic in range(2)]
                for dt_ in range(DT):
                    lhs = wT[:, dt_, et * 128:(et + 1) * 128]
                    for ic in range(2):
                        nc.tensor.matmul(
                            pss[ic], lhs,
                            xT[:, dt_, (2 * hf + ic) * 512:
                               (2 * hf + ic + 1) * 512],
                            start=(dt_ == 0), stop=(dt_ == DT - 1))
                for ic in range(2):
                    nc.any.tensor_scalar_add(
                        out=dst[:, et, (2 * hf + ic) * 512:
                                (2 * hf + ic + 1) * 512],
                        in0=pss[ic], scalar1=b_sb[:, et:et + 1])

        for w_d_, wT_, x_d_, xT_, xt_sb_, b_sb_, nm_ in (
                (wq_d, wqT, q_d, QT, qT, qb_sb, "q"),
                (wk_d, wkT, k_d, KT, kT, kb_sb, "k")):
            for et in range(0, DT, 2):
                transpose_in(wT_, w_d_, et, 2, f"w{nm_}")
            for it in range(0, LT // 2, 2):
                transpose_in(xT_, x_d_, it, 2, nm_)
            project_half(xt_sb_, wT_, xT_, b_sb_, nm_, 0)
            for it in range(LT // 2, LT, 2):
                transpose_in(xT_, x_d_, it, 2, nm_)
            project_half(xt_sb_, wT_, xT_, b_sb_, nm_, 1)
        pool_xt.release()
        pool_w.release()

        # ---- V load (overlaps stage D) ---------------------------------
        pool_v = tc.alloc_tile_pool(name="pool_v", bufs=1)
        V_sb = pool_v.tile([128, LT, D], bf16, name="V_sb")
        for jt in range(LT):
            nc.gpsimd.dma_start(out=V_sb[:, jt, :],
                                in_=v_d[jt * 128:(jt + 1) * 128, :])

        # ---- stage D: S^T strips + exp -> P^T [j, i] bf16 --------------
        pool_pt = tc.alloc_tile_pool(name="pool_pt", bufs=1)
        PT = pool_pt.tile([128, LT, L], bf16, name="PT")
        for jt in range(LT):
            pss = [psum.tile([128, 512], f32, tag="mm", bufs=6,
                             name=f"s_{jt}_{ic}") for ic in range(4)]
            for et in range(DT):
                lhs = kT[:, et, jt * 128:(jt + 1) * 128]
                for ic in range(4):
                    nc.tensor.matmul(pss[ic], lhs,
                                     qT[:, et, ic * 512:(ic + 1) * 512],
                                     start=(et == 0), stop=(et == DT - 1))
            for ic in range(4):
                nc.scalar.activation(out=PT[:, jt, ic * 512:(ic + 1) * 512],
                                     in_=pss[ic], func=AF.Exp,
                                     bias=bias_b[:, jt:jt + 1], scale=SCALE)
        pool_qkt.release()

        # ---- stage E: O = P^T.T @ V, denominator, normalize, store -----
        for it in range(LT):
            o_lo = psum.tile([128, 512], f32, tag="mm", bufs=6,
                             name=f"o_lo_{it}")
            o_hi = psum.tile([128, 512], f32, tag="mm", bufs=6,
                             name=f"o_hi_{it}")
            d_ps = psum.tile([128, 1], f32, tag="d", bufs=2,
                             name=f"d_ps_{it}")
            for jt in range(LT):
                lhs = PT[:, jt, it * 128:(it + 1) * 128]
                # denominator first: its matmul is tiny, so the (hidden)
                # weight load is immediately followed by dense N=512 work
                nc.tensor.matmul(d_ps, lhs, ones_bf,
                                 start=(jt == 0), stop=(jt == LT - 1))
                nc.tensor.matmul(o_lo, lhs, V_sb[:, jt, 0:512],
                                 start=(jt == 0), stop=(jt == LT - 1))
                nc.tensor.matmul(o_hi, lhs, V_sb[:, jt, 512:1024],
                                 start=(jt == 0), stop=(jt == LT - 1))
            recip = stage.tile([128, 1], f32, tag="recip", bufs=4,
                               name=f"recip_{it}")
            nc.vector.reciprocal(recip, d_ps)
            o_sb = stage.tile([128, D], f32, tag="osb", bufs=3,
                              name=f"o_sb_{it}")
            # normalize + store per half so the first store overlaps the
            # second normalize (shrinks the end-of-kernel drain)
            nc.any.tensor_scalar_mul(o_sb[:, 0:512], o_lo, recip)
            nc.sync.dma_start(out=o_d[it * 128:(it + 1) * 128, 0:512],
                              in_=o_sb[:, 0:512])
            nc.any.tensor_scalar_mul(o_sb[:, 512:1024], o_hi, recip)
            nc.sync.dma_start(out=o_d[it * 128:(it + 1) * 128, 512:1024],
                              in_=o_sb[:, 512:1024])
        pool_pt.release()
        pool_v.release()

        stage.release()
        psum.release()
        singles.release()

    nc.compile()
    return nc


def _get_program():
    if "nc" not in _cache:
        _cache["nc"] = _build_program()
    return _cache["nc"]


def _enable_jax_cache():
    try:
        import jax
        jax.config.update("jax_compilation_cache_dir",
                          "/tmp/jax_neff_cache")
        jax.config.update("jax_persistent_cache_min_compile_time_secs", 10.0)
        jax.config.update("jax_persistent_cache_min_entry_size_bytes", -1)
    except Exception:
        pass
    _install_neff_cache()


def _install_neff_cache():
    """Cache compiled NEFFs on disk keyed by BIR hash: the walrus backend
    compile is 10s-5min per fresh process and fully deterministic."""
    try:
        import hashlib
        import os
        import shutil
        from concourse import bass2jax, bass_utils
        if getattr(bass_utils, "_ant_neff_cache_installed", False):
            return
        orig = bass_utils.compile_bir_kernel
        cache_dir = "/tmp/bass_neff_cache"

        def cached(bir_json, tmpdir, neff_name="file.neff"):
            key = hashlib.sha256(bir_json).hexdigest()[:32]
            hit = os.path.join(cache_dir, key + ".neff")
            try:
                if os.path.exists(hit):
                    dst = os.path.join(tmpdir, neff_name)
                    shutil.copy(hit, dst)
                    return dst
            except Exception:
                pass
            neff_path = orig(bir_json, tmpdir, neff_name)
            try:
                os.makedirs(cache_dir, exist_ok=True)
                shutil.copy(neff_path, hit + ".tmp")
                os.replace(hit + ".tmp", hit)
            except Exception:
                pass
            return neff_path

        bass_utils.compile_bir_kernel = cached
        bass2jax.compile_bir_kernel = cached
        bass_utils._ant_neff_cache_installed = True
    except Exception:
        pass


def kernel(query, key, value, mask, Wq_w, Wq_b, Wk_w, Wk_b):
    from concourse.bass_utils import run_bass_kernel_spmd

    _enable_jax_cache()
    nc = _get_program()
    in_maps = []
    for c in range(N_CORES):
        in_maps.append({
            "query": np.ascontiguousarray(query[c], dtype=np.float32),
            "key": np.ascontiguousarray(key[c], dtype=np.float32),
            "value": np.ascontiguousarray(value[c], dtype=np.float32),
            "mask": np.ascontiguousarray(mask[c], dtype=np.int32),
            "Wq_w": np.ascontiguousarray(Wq_w, dtype=np.float32),
            "Wq_b": np.ascontiguousarray(Wq_b, dtype=np.float32),
            "Wk_w": np.ascontiguousarray(Wk_w, dtype=np.float32),
            "Wk_b": np.ascontiguousarray(Wk_b, dtype=np.float32),
        })
    res = run_bass_kernel_spmd(nc, in_maps, list(range(N_CORES)))
    return np.stack([res.results[c]["out"] for c in range(N_CORES)], axis=0)
